# revision 3
# baseline (speedup 1.0000x reference)
"""Causal self-attention (B=2, T=2048, D=1024, H=16, Dh=64) on 8 TRN2 cores.

Sharding: core c = 4*b + g -> batch b (data parallel), head group g of 4
heads (tensor parallel on heads for Wq/Wk/Wv, column-split of the proj
input with the resulting partial-sum reduction done host-side at unshard).

Per-core dataflow (all layouts chosen so no on-device transposes happen):
  qT,kT [256, 2048] = W{q,k}_g @ x.T      (lhsT = W{q,k}_g.T from host)
  v     [t-block 128, 4 heads x (64 v | 64 ones)] bf16
  attention, transposed: PT[tk, tq] = kT_h.T-block @ qT_h, exp on ACT,
  causal mask as a post-exp 0/1 multiply on the diagonal 128-block,
  AV: yT[d, tq] (+ softmax column sums for free via the ones columns of v)
  normalize: yT * recip(sums) -> ytsb [256, 2048] (proj lhsT layout)
  proj partial: out[t, :] = ytsb.T-block @ Wp_gT
Host: out[b] = sum_g partial[4b+g] + bp.

Matmuls run as float32r (full-rate reduced-precision fp32) except the
attention P/V matmuls which are bf16.
"""

import numpy as np

import concourse.bass as bass
import concourse.mybir as mybir
import concourse.tile as tile
from concourse import bacc
from concourse import bass_utils

F32 = mybir.dt.float32
F32R = mybir.dt.float32r
BF16 = mybir.dt.bfloat16

B, T, D = 2, 2048, 1024
H, DH = 16, 64
N_CORES = 8
HPC = 4            # heads per core
GD = HPC * DH      # 256 feature cols per core
KT = D // 128      # 8 k-tiles over the model dim
TB = T // 128      # 16 t-blocks of 128
NEG = 0.125        # logit scale 1/sqrt(Dh)

_cache = {}


def _build():
    nc = bacc.Bacc("TRN2", target_bir_lowering=False, debug=False,
                   num_devices=N_CORES)

    xT_d = nc.dram_tensor("xT", [D, T], F32R, kind="ExternalInput")
    wqT_d = nc.dram_tensor("wqT", [D, GD], F32R, kind="ExternalInput")
    wkT_d = nc.dram_tensor("wkT", [D, GD], F32R, kind="ExternalInput")
    wvT_d = nc.dram_tensor("wvT", [D, GD], F32R, kind="ExternalInput")
    wpT_d = nc.dram_tensor("wpT", [GD, D], F32R, kind="ExternalInput")
    bq_d = nc.dram_tensor("bq2", [128, 2], F32, kind="ExternalInput")
    bk_d = nc.dram_tensor("bk2", [128, 2], F32, kind="ExternalInput")
    bvb_d = nc.dram_tensor("bvb", [128, GD], F32, kind="ExternalInput")
    msk_d = nc.dram_tensor("mask01", [128, 128], BF16, kind="ExternalInput")
    out_d = nc.dram_tensor("out", [T, D], F32, kind="ExternalOutput")

    with tile.TileContext(nc) as tc:
        with (
            tc.tile_pool(name="const", bufs=1) as cp,
            tc.tile_pool(name="big", bufs=1) as bp_,
            tc.tile_pool(name="work", bufs=3) as wp_,
            tc.tile_pool(name="outp", bufs=3) as op_,
            tc.tile_pool(name="pA", bufs=2, space="PSUM") as pA,
            tc.tile_pool(name="pB", bufs=2, space="PSUM") as pB,
        ):
            # ---- loads ----
            xt = []
            for k in range(KT):
                t_ = cp.tile([128, T], F32R, tag=f"xt{k}", name=f"xt{k}")
                nc.sync.dma_start(t_[:], xT_d[k * 128:(k + 1) * 128, :])
                xt.append(t_)
            wq = cp.tile([128, KT, GD], F32R, tag="wq", name="wq")
            wk = cp.tile([128, KT, GD], F32R, tag="wk", name="wk")
            wv = cp.tile([128, KT, GD], F32R, tag="wv", name="wv")
            nc.sync.dma_start(wq[:], wqT_d.rearrange("(a p) m -> p a m", p=128))
            nc.sync.dma_start(wk[:], wkT_d.rearrange("(a p) m -> p a m", p=128))
            nc.sync.dma_start(wv[:], wvT_d.rearrange("(a p) m -> p a m", p=128))
            wpt = []
            for p in range(2):
                t_ = cp.tile([128, D], F32R, tag=f"wp{p}", name=f"wp{p}")
                nc.sync.dma_start(t_[:], wpT_d[p * 128:(p + 1) * 128, :])
                wpt.append(t_)
            bq2 = cp.tile([128, 2], F32, tag="bq2", name="bq2")
            bk2 = cp.tile([128, 2], F32, tag="bk2", name="bk2")
            bvb = cp.tile([128, GD], F32, tag="bvb", name="bvb")
            msk = cp.tile([128, 128], BF16, tag="msk", name="msk")
            nc.sync.dma_start(bq2[:], bq_d[:])
            nc.sync.dma_start(bk2[:], bk_d[:])
            nc.sync.dma_start(bvb[:], bvb_d[:])
            nc.sync.dma_start(msk[:], msk_d[:])

            # ---- q/k projections -> qt/kt [2 x (128, 2048)] ----
            qt = [bp_.tile([128, T], F32R, tag=f"qt{m}", name=f"qt{m}") for m in range(2)]
            kt = [bp_.tile([128, T], F32R, tag=f"kt{m}", name=f"kt{m}") for m in range(2)]
            for dst, w, b2 in ((qt, wq, bq2), (kt, wk, bk2)):
                for m in range(2):
                    for n in range(4):
                        pool = (pA, pB)[(m * 4 + n) % 2]
                        ps = pool.tile([128, 1024], F32, tag=pool.name,
                                       name="psqk")
                        for k in range(KT):
                            nc.tensor.matmul(
                                ps[:, 0:512],
                                w[:, k, m * 128:(m + 1) * 128],
                                xt[k][:, n * 512:(n + 1) * 512],
                                start=(k == 0), stop=(k == KT - 1),
                            )
                        nc.vector.tensor_scalar_add(
                            dst[m][:, n * 512:(n + 1) * 512], ps[:, 0:512],
                            b2[:, m:m + 1],
                        )

            # ---- v projection -> per t-block [128, 4*(64 v | 64 ones)] bf16
            vt = []
            for t in range(TB):
                vtile = bp_.tile([128, 4, 2, DH], BF16, tag=f"v{t}", name=f"v{t}")
                nc.gpsimd.memset(vtile[:, :, 1, :], 1.0)
                pool = (pA, pB)[t % 2]
                ps = pool.tile([128, 1024], F32, tag=pool.name, name="psv")
                for k in range(KT):
                    nc.tensor.matmul(
                        ps[:, 0:GD],
                        xt[k][:, t * 128:(t + 1) * 128],
                        wv[:, k, :],
                        start=(k == 0), stop=(k == KT - 1),
                    )
                nc.vector.tensor_add(
                    vtile[:, :, 0, :],
                    ps[:, 0:GD].rearrange("p (h d) -> p h d", h=4),
                    bvb.rearrange("p (h d) -> p h d", h=4),
                )
                vt.append(vtile)

            # ---- attention -> ytsb [2 x (128, 2048)] (proj lhsT layout) ----
            ytsb = [bp_.tile([128, T], F32R, tag=f"yt{p}", name=f"yt{p}") for p in range(2)]
            for Ti in range(2):          # tq cols [1024*Ti, 1024*Ti+1024)
                for hp in range(2):      # head pair -> qt/kt tile hp
                    ytp = [pB.tile([128, 1024], F32, tag=pB.name, name="psyt")
                           for _ in range(2)]
                    last = 8 * (Ti + 1) - 1
                    for tkb in range(8 * (Ti + 1)):
                        s = max(0, 128 * tkb - 1024 * Ti)
                        for j in range(2):
                            h = 2 * hp + j
                            pt = pA.tile([128, 1024], F32, tag=pA.name, name="pspt")
                            for bk in range(2):
                                c0, c1 = max(s, 512 * bk), 512 * (bk + 1)
                                if c0 >= c1:
                                    continue
                                nc.tensor.matmul(
                                    pt[:, c0:c1],
                                    kt[hp][64 * j:64 * j + 64,
                                           128 * tkb:128 * (tkb + 1)],
                                    qt[hp][64 * j:64 * j + 64,
                                           1024 * Ti + c0:1024 * Ti + c1],
                                    start=True, stop=True,
                                )
                            ptsb = wp_.tile([128, 1024], BF16, tag="ptsb", name="ptsb")
                            nc.scalar.activation(
                                ptsb[:, s:1024], pt[:, s:1024],
                                mybir.ActivationFunctionType.Exp, scale=NEG,
                            )
                            if 128 * tkb >= 1024 * Ti:  # diagonal block
                                nc.vector.tensor_mul(
                                    ptsb[:, s:s + 128], ptsb[:, s:s + 128],
                                    msk[:],
                                )
                            for bk in range(2):
                                c0, c1 = max(s, 512 * bk), 512 * (bk + 1)
                                if c0 >= c1:
                                    continue
                                nc.tensor.matmul(
                                    ytp[j][:, c0:c1],
                                    vt[tkb][:, h, :, :].rearrange(
                                        "p a d -> p (a d)"),
                                    ptsb[:, c0:c1],
                                    start=(tkb == 0), stop=(tkb == last),
                                )
                    for j in range(2):
                        rc = wp_.tile([64, 1024], F32, tag="recip", name="recip")
                        nc.vector.reciprocal(rc[:], ytp[j][64:128, :])
                        nc.vector.tensor_mul(
                            ytsb[hp][64 * j:64 * j + 64,
                                     1024 * Ti:1024 * (Ti + 1)],
                            ytp[j][0:64, :], rc[:],
                        )

            # ---- output projection (partial) ----
            for t in range(TB):
                pool = (pA, pB)[t % 2]
                po = pool.tile([128, 1024], F32, tag=pool.name, name="pso")
                for n in range(2):
                    for p in range(2):
                        nc.tensor.matmul(
                            po[:, 512 * n:512 * (n + 1)],
                            ytsb[p][:, 128 * t:128 * (t + 1)],
                            wpt[p][:, 512 * n:512 * (n + 1)],
                            start=(p == 0), stop=(p == 1),
                        )
                ob = op_.tile([128, 1024], F32, tag="ob", name="ob")
                nc.scalar.copy(ob[:], po[:])
                nc.sync.dma_start(out_d[128 * t:128 * (t + 1), :], ob[:])

    nc.compile()
    return nc


def _shard(x, Wq, bq, Wk, bk, Wv, bv, Wp, bp):
    f32 = np.float32
    import ml_dtypes
    mask01 = np.triu(np.ones((128, 128), f32)).astype(ml_dtypes.bfloat16)
    in_maps = []
    for c in range(N_CORES):
        b, g = divmod(c, HPC)
        sl = slice(GD * g, GD * (g + 1))
        in_maps.append({
            "xT": np.ascontiguousarray(x[b].T, dtype=f32),
            "wqT": np.ascontiguousarray(Wq[sl, :].T, dtype=f32),
            "wkT": np.ascontiguousarray(Wk[sl, :].T, dtype=f32),
            "wvT": np.ascontiguousarray(Wv[sl, :].T, dtype=f32),
            "wpT": np.ascontiguousarray(Wp[:, sl].T, dtype=f32),
            "bq2": np.ascontiguousarray(bq[sl].reshape(2, 128).T, dtype=f32),
            "bk2": np.ascontiguousarray(bk[sl].reshape(2, 128).T, dtype=f32),
            "bvb": np.broadcast_to(bv[sl], (128, GD)).astype(f32),
            "mask01": mask01,
        })
    return in_maps


def run(inputs, trace=False):
    """Run the SPMD kernel; returns (output [B,T,D] f32, BassKernelResults)."""
    if "nc" not in _cache:
        _cache["nc"] = _build()
    nc = _cache["nc"]
    in_maps = _shard(**inputs)
    if trace:
        _install_ntff_hook()
    res = bass_utils.run_bass_kernel_spmd(
        nc, in_maps, core_ids=list(range(N_CORES)), trace=trace,
    )
    bp = np.asarray(inputs["bp"], dtype=np.float32)
    out = np.empty((B, T, D), dtype=np.float32)
    for b in range(B):
        acc = res.results[4 * b]["out"].astype(np.float32)
        for g in range(1, HPC):
            acc = acc + res.results[4 * b + g]["out"]
        out[b] = acc + bp
    return out, res


def kernel(**inputs):
    out, _ = run(inputs, trace=False)
    return out


def _install_ntff_hook():
    """antenv.axon_hooks is absent on this image; inject it so
    run_bass_kernel_spmd(trace=True) can capture NTFF profiles."""
    import sys, types
    if "antenv.axon_hooks" in sys.modules:
        return
    try:
        mod = types.ModuleType("antenv.axon_hooks")
        mod._hook = None
        mod.set_axon_ntff_profile_hook = lambda h: setattr(mod, "_hook", h)
        mod.get_axon_ntff_profile_hook = lambda: mod._hook
        sys.modules["antenv.axon_hooks"] = mod
        import antenv
        antenv.axon_hooks = mod
        from trn_agent_boot.trn_boot import _ntff_profile_via_ctypes
        mod.set_axon_ntff_profile_hook(
            _ntff_profile_via_ctypes("/opt/axon/libaxon_pjrt.so"))
    except Exception:
        pass
